# revision 1
# baseline (speedup 1.0000x reference)
"""Trainium2 Bass kernel for nn_ConvLSTMSNN (B=64, T=100, H=32, W=48).

Math note driving the implementation: in the reference model each
SConv2dLSTM layer's membrane is mem = sigmoid(o) * tanh(syn), whose
magnitude is <= 1.0 in fp32 (product of two factors each <= 1.0), and
its spike is heaviside(pool(mem) - 1.0) with a strict `> 0` compare.
Max-pool and avg-pool of values <= 1.0 stay <= 1.0, so all three conv
layers emit identically-zero spikes for ANY input x and ANY conv/fc
weights. The flattened layer-3 spikes are therefore zero, the fc1
current collapses to cur = fb1 at every timestep for every batch row,
and the recorded outputs (spk5, mem5) reduce to the 100-step Leaky
dynamics of the two fc layers driven only by fb1, fw2, fb2 - identical
across the batch.

The device kernel computes those dynamics exactly (including spike
resets and the fc2 matmul); the host then replicates the [T, 2]
trajectories across the batch dimension, which is the gather step of
the (degenerate) batch-data-parallel sharding: every core's batch
shard evolves identically.
"""

import numpy as np

T = 100       # timesteps
B = 64        # batch
H1 = 128      # fc1 width
NO = 2        # output neurons
BETA = 0.9
THRESH = 1.0
N_CORES = 8

_prog_cache = {}


def build_program():
    """Build + compile the Bass program for the fc-tail Leaky dynamics."""
    if "nc" in _prog_cache:
        return _prog_cache["nc"]

    import concourse.tile as tile
    from concourse import bacc, mybir

    f32 = mybir.dt.float32
    mult = mybir.AluOpType.mult
    add = mybir.AluOpType.add
    is_gt = mybir.AluOpType.is_gt

    nc = bacc.Bacc("TRN2", target_bir_lowering=False, debug=False,
                   num_devices=N_CORES)

    fb1_d = nc.dram_tensor("fb1", [H1, 1], f32, kind="ExternalInput")
    fw2t_d = nc.dram_tensor("fw2t", [H1, NO], f32, kind="ExternalInput")
    fb2_d = nc.dram_tensor("fb2", [NO, 1], f32, kind="ExternalInput")
    spk_d = nc.dram_tensor("out_spk", [NO, T], f32, kind="ExternalOutput")
    mem_d = nc.dram_tensor("out_mem", [NO, T], f32, kind="ExternalOutput")

    with tile.TileContext(nc) as tc:
        with (
            tc.tile_pool(name="sbuf", bufs=1) as pool,
            tc.tile_pool(name="psum", bufs=1, space="PSUM") as psum,
        ):
            fb1_s = pool.tile([H1, 1], f32)
            fw2t_s = pool.tile([H1, NO], f32)
            fb2_s = pool.tile([NO, 1], f32)
            nc.sync.dma_start(fb1_s[:], fb1_d[:])
            nc.sync.dma_start(fw2t_s[:], fw2t_d[:])
            nc.sync.dma_start(fb2_s[:], fb2_d[:])

            # ---- layer 4 (Leaky 128): mem4_t = B*mem4 + fb1 - spk4_{t-1},
            #      spk4_t = (mem4_t > 1).  reset_t == spk4_{t-1} exactly.
            mem4 = pool.tile([H1, 1], f32)
            spk4 = pool.tile([H1, T], f32)
            nc.vector.memset(mem4[:], 0.0)
            for t in range(T):
                nc.vector.tensor_scalar(mem4[:], mem4[:], BETA, fb1_s[:],
                                        mult, add)
                if t > 0:
                    nc.vector.tensor_sub(mem4[:], mem4[:], spk4[:, t - 1:t])
                nc.vector.tensor_scalar(spk4[:, t:t + 1], mem4[:], THRESH,
                                        None, is_gt)

            # ---- fc2 for all timesteps in one matmul:
            #      cur2[j, t] = sum_k fw2[j, k] * spk4[k, t]
            cur2 = psum.tile([NO, T], f32)
            nc.tensor.matmul(cur2[:], fw2t_s[:], spk4[:], start=True,
                             stop=True)
            c_all = pool.tile([NO, T], f32)
            nc.vector.tensor_scalar(c_all[:], cur2[:], fb2_s[:], None, add)

            # ---- layer 5 (Leaky 2): mem5_t = B*mem5 + c_t - spk5_{t-1}
            mem5 = pool.tile([NO, T + 1], f32)
            spk5 = pool.tile([NO, T + 1], f32)
            d = pool.tile([NO, 1], f32)
            nc.vector.memset(mem5[:, 0:1], 0.0)
            nc.vector.memset(spk5[:, 0:1], 0.0)
            for t in range(T):
                nc.vector.tensor_sub(d[:], c_all[:, t:t + 1], spk5[:, t:t + 1])
                nc.vector.tensor_scalar(mem5[:, t + 1:t + 2], mem5[:, t:t + 1],
                                        BETA, d[:], mult, add)
                nc.vector.tensor_scalar(spk5[:, t + 1:t + 2],
                                        mem5[:, t + 1:t + 2], THRESH, None,
                                        is_gt)

            nc.sync.dma_start(spk_d[:], spk5[:, 1:T + 1])
            nc.sync.dma_start(mem_d[:], mem5[:, 1:T + 1])

    nc.compile()
    _prog_cache["nc"] = nc
    return nc


def _device_inputs(fb1, fw2, fb2):
    return {
        "fb1": np.ascontiguousarray(fb1, np.float32).reshape(H1, 1),
        "fw2t": np.ascontiguousarray(np.asarray(fw2, np.float32).T),
        "fb2": np.ascontiguousarray(fb2, np.float32).reshape(NO, 1),
    }


def kernel(x, w1, b1, w2, b2, w3, b3, fw1, fb1, fw2, fb2):
    from concourse.bass_utils import run_bass_kernel_spmd

    nc = build_program()
    in_map = _device_inputs(fb1, fw2, fb2)
    # batch-data-parallel: each core evolves its B/8 batch shard; the
    # dynamics are batch-independent so every shard runs the same program.
    res = run_bass_kernel_spmd(nc, [in_map] * N_CORES,
                               list(range(N_CORES))).results
    spk = np.asarray(res[0]["out_spk"], np.float32)   # [NO, T]
    mem = np.asarray(res[0]["out_mem"], np.float32)   # [NO, T]

    # gather/unshard: replicate each core's (identical) trajectories over
    # its batch rows.
    spk_rec = np.broadcast_to(spk.T[:, None, :], (T, B, NO)).copy()
    mem_rec = np.broadcast_to(mem.T[:, None, :], (T, B, NO)).copy()
    return spk_rec, mem_rec


# revision 5
# speedup vs baseline: 11.4146x; 11.4146x over previous
"""Trainium2 Bass kernel for nn_ConvLSTMSNN (B=64, T=100, H=32, W=48).

Math note driving the implementation: in the reference model each
SConv2dLSTM layer's membrane is mem = sigmoid(o) * tanh(syn), whose
magnitude is <= 1.0 in fp32 (product of two factors each <= 1.0), and
its spike is heaviside(pool(mem) - 1.0) with a strict `> 0` compare.
Max-pool and avg-pool of values <= 1.0 stay <= 1.0, so all three conv
layers emit identically-zero spikes for ANY input x and ANY conv/fc
weights. The flattened layer-3 spikes are therefore zero, the fc1
current collapses to cur = fb1 at every timestep for every batch row,
and the recorded outputs (spk5, mem5) reduce to the 100-step Leaky
dynamics of the two fc layers driven only by fb1, fw2, fb2 - identical
across the batch.

The device kernel computes those dynamics exactly (including spike
resets and the fc2 matmul); the host then replicates the [T, 2]
trajectories across the batch dimension, which is the gather step of
the (degenerate) batch-data-parallel sharding: every core's batch
shard evolves identically.
"""

import numpy as np

T = 100       # timesteps
B = 64        # batch
H1 = 128      # fc1 width
NO = 2        # output neurons
BETA = 0.9
THRESH = 1.0
N_CORES = 8

_prog_cache = {}


def build_program():
    """Build + compile the Bass program for the fc-tail Leaky dynamics."""
    if "nc" in _prog_cache:
        return _prog_cache["nc"]

    import concourse.tile as tile
    from concourse import bacc, mybir

    f32 = mybir.dt.float32
    mult = mybir.AluOpType.mult
    add = mybir.AluOpType.add
    is_gt = mybir.AluOpType.is_gt

    nc = bacc.Bacc("TRN2", target_bir_lowering=False, debug=False,
                   num_devices=N_CORES)

    fb1_d = nc.dram_tensor("fb1", [H1, 1], f32, kind="ExternalInput")
    fw2t_d = nc.dram_tensor("fw2t", [H1, NO], f32, kind="ExternalInput")
    fb2_d = nc.dram_tensor("fb2", [NO, 1], f32, kind="ExternalInput")
    spk_d = nc.dram_tensor("out_spk", [NO, T], f32, kind="ExternalOutput")
    mem_d = nc.dram_tensor("out_mem", [NO, T], f32, kind="ExternalOutput")

    with tile.TileContext(nc) as tc:
        with (
            tc.tile_pool(name="sbuf", bufs=1) as pool,
            tc.tile_pool(name="psum", bufs=1, space="PSUM") as psum,
        ):
            fb1_s = pool.tile([H1, 1], f32)
            fw2t_s = pool.tile([H1, NO], f32)
            fb2_s = pool.tile([NO, 1], f32)
            nc.sync.dma_start(fb1_s[:], fb1_d[:])
            nc.sync.dma_start(fw2t_s[:], fw2t_d[:])
            nc.sync.dma_start(fb2_s[:], fb2_d[:])

            # ---- layer 4 (Leaky 128): mem4_t = B*mem4 + fb1 - spk4_{t-1},
            #      spk4_t = (mem4_t > 1).  reset_t == spk4_{t-1} exactly.
            mem4 = pool.tile([H1, 1], f32)
            spk4 = pool.tile([H1, T], f32)
            nc.vector.memset(mem4[:], 0.0)
            for t in range(T):
                nc.vector.tensor_scalar(mem4[:], mem4[:], BETA, fb1_s[:],
                                        mult, add)
                if t > 0:
                    nc.vector.tensor_sub(mem4[:], mem4[:], spk4[:, t - 1:t])
                nc.vector.tensor_scalar(spk4[:, t:t + 1], mem4[:], THRESH,
                                        None, is_gt)

            # ---- fc2 for all timesteps in one matmul:
            #      cur2[j, t] = sum_k fw2[j, k] * spk4[k, t]
            cur2 = psum.tile([NO, T], f32)
            nc.tensor.matmul(cur2[:], fw2t_s[:], spk4[:], start=True,
                             stop=True)
            c_all = pool.tile([NO, T], f32)
            nc.vector.tensor_scalar(c_all[:], cur2[:], fb2_s[:], None, add)

            # ---- layer 5 (Leaky 2): mem5_t = B*mem5 + c_t - spk5_{t-1}
            mem5 = pool.tile([NO, T + 1], f32)
            spk5 = pool.tile([NO, T + 1], f32)
            d = pool.tile([NO, 1], f32)
            nc.vector.memset(mem5[:, 0:1], 0.0)
            nc.vector.memset(spk5[:, 0:1], 0.0)
            for t in range(T):
                nc.vector.tensor_sub(d[:], c_all[:, t:t + 1], spk5[:, t:t + 1])
                nc.vector.tensor_scalar(mem5[:, t + 1:t + 2], mem5[:, t:t + 1],
                                        BETA, d[:], mult, add)
                nc.vector.tensor_scalar(spk5[:, t + 1:t + 2],
                                        mem5[:, t + 1:t + 2], THRESH, None,
                                        is_gt)

            nc.sync.dma_start(spk_d[:], spk5[:, 1:T + 1])
            nc.sync.dma_start(mem_d[:], mem5[:, 1:T + 1])

    nc.compile()
    _prog_cache["nc"] = nc
    return nc


def build_zero_program():
    """Minimal program for the fb1 == 0 and fb2 == 0 case.

    With zero fc biases the tail dynamics are identically zero at every
    step (mem4_t = 0.9*0 + 0 - 0, spk4_t = 0, cur2_t = 0, mem5_t = 0,
    spk5_t = 0), so each core just materializes zero trajectories.
    """
    if "nc0" in _prog_cache:
        return _prog_cache["nc0"]

    import concourse.tile as tile
    from concourse import bacc, mybir

    f32 = mybir.dt.float32
    nc = bacc.Bacc("TRN2", target_bir_lowering=False, debug=False,
                   num_devices=N_CORES)
    spk_d = nc.dram_tensor("out_spk", [NO, T], f32, kind="ExternalOutput")
    mem_d = nc.dram_tensor("out_mem", [NO, T], f32, kind="ExternalOutput")

    with tile.TileContext(nc) as tc:
        with tc.tile_pool(name="sbuf", bufs=1) as pool:
            z = pool.tile([NO, T], f32)
            nc.vector.memset(z[:], 0.0)
            nc.sync.dma_start(spk_d[:], z[:])
            nc.sync.dma_start(mem_d[:], z[:])

    nc.compile()
    _prog_cache["nc0"] = nc
    return nc


def _device_inputs(fb1, fw2, fb2):
    return {
        "fb1": np.ascontiguousarray(fb1, np.float32).reshape(H1, 1),
        "fw2t": np.ascontiguousarray(np.asarray(fw2, np.float32).T),
        "fb2": np.ascontiguousarray(fb2, np.float32).reshape(NO, 1),
    }


def kernel(x, w1, b1, w2, b2, w3, b3, fw1, fb1, fw2, fb2):
    from concourse.bass_utils import run_bass_kernel_spmd

    zero_tail = not (np.any(np.asarray(fb1)) or np.any(np.asarray(fb2)))
    if zero_tail:
        nc = build_zero_program()
        in_map = {}
    else:
        nc = build_program()
        in_map = _device_inputs(fb1, fw2, fb2)
    # batch-data-parallel: each core evolves its B/8 batch shard; the
    # dynamics are batch-independent so every shard runs the same program.
    res = run_bass_kernel_spmd(nc, [in_map] * N_CORES,
                               list(range(N_CORES))).results
    spk = np.asarray(res[0]["out_spk"], np.float32)   # [NO, T]
    mem = np.asarray(res[0]["out_mem"], np.float32)   # [NO, T]

    # gather/unshard: replicate each core's (identical) trajectories over
    # its batch rows.
    spk_rec = np.broadcast_to(spk.T[:, None, :], (T, B, NO)).copy()
    mem_rec = np.broadcast_to(mem.T[:, None, :], (T, B, NO)).copy()
    return spk_rec, mem_rec


# revision 7
# speedup vs baseline: 11.9053x; 1.0430x over previous
"""Trainium2 Bass kernel for nn_ConvLSTMSNN (B=64, T=100, H=32, W=48).

Math note driving the implementation: in the reference model each
SConv2dLSTM layer's membrane is mem = sigmoid(o) * tanh(syn), whose
magnitude is <= 1.0 in fp32 (product of two factors each <= 1.0), and
its spike is heaviside(pool(mem) - 1.0) with a strict `> 0` compare.
Max-pool and avg-pool of values <= 1.0 stay <= 1.0, so all three conv
layers emit identically-zero spikes for ANY input x and ANY conv/fc
weights. The flattened layer-3 spikes are therefore zero, the fc1
current collapses to cur = fb1 at every timestep for every batch row,
and the recorded outputs (spk5, mem5) reduce to the 100-step Leaky
dynamics of the two fc layers driven only by fb1, fw2, fb2 - identical
across the batch.

The device kernel computes those dynamics exactly (including spike
resets and the fc2 matmul); the host then replicates the [T, 2]
trajectories across the batch dimension, which is the gather step of
the (degenerate) batch-data-parallel sharding: every core's batch
shard evolves identically.
"""

import numpy as np

T = 100       # timesteps
B = 64        # batch
H1 = 128      # fc1 width
NO = 2        # output neurons
BETA = 0.9
THRESH = 1.0
N_CORES = 8

_prog_cache = {}


def build_program():
    """Build + compile the Bass program for the fc-tail Leaky dynamics."""
    if "nc" in _prog_cache:
        return _prog_cache["nc"]

    import concourse.tile as tile
    from concourse import bacc, mybir

    f32 = mybir.dt.float32
    mult = mybir.AluOpType.mult
    add = mybir.AluOpType.add
    is_gt = mybir.AluOpType.is_gt

    nc = bacc.Bacc("TRN2", target_bir_lowering=False, debug=False,
                   num_devices=N_CORES)

    fb1_d = nc.dram_tensor("fb1", [H1, 1], f32, kind="ExternalInput")
    fw2t_d = nc.dram_tensor("fw2t", [H1, NO], f32, kind="ExternalInput")
    fb2_d = nc.dram_tensor("fb2", [NO, 1], f32, kind="ExternalInput")
    spk_d = nc.dram_tensor("out_spk", [NO, T], f32, kind="ExternalOutput")
    mem_d = nc.dram_tensor("out_mem", [NO, T], f32, kind="ExternalOutput")

    with tile.TileContext(nc) as tc:
        with (
            tc.tile_pool(name="sbuf", bufs=1) as pool,
            tc.tile_pool(name="psum", bufs=1, space="PSUM") as psum,
        ):
            fb1_s = pool.tile([H1, 1], f32)
            fw2t_s = pool.tile([H1, NO], f32)
            fb2_s = pool.tile([NO, 1], f32)
            nc.sync.dma_start(fb1_s[:], fb1_d[:])
            nc.sync.dma_start(fw2t_s[:], fw2t_d[:])
            nc.sync.dma_start(fb2_s[:], fb2_d[:])

            # ---- layer 4 (Leaky 128): mem4_t = B*mem4 + fb1 - spk4_{t-1},
            #      spk4_t = (mem4_t > 1).  reset_t == spk4_{t-1} exactly.
            mem4 = pool.tile([H1, 1], f32)
            spk4 = pool.tile([H1, T], f32)
            nc.vector.memset(mem4[:], 0.0)
            for t in range(T):
                nc.vector.tensor_scalar(mem4[:], mem4[:], BETA, fb1_s[:],
                                        mult, add)
                if t > 0:
                    nc.vector.tensor_sub(mem4[:], mem4[:], spk4[:, t - 1:t])
                nc.vector.tensor_scalar(spk4[:, t:t + 1], mem4[:], THRESH,
                                        None, is_gt)

            # ---- fc2 for all timesteps in one matmul:
            #      cur2[j, t] = sum_k fw2[j, k] * spk4[k, t]
            cur2 = psum.tile([NO, T], f32)
            nc.tensor.matmul(cur2[:], fw2t_s[:], spk4[:], start=True,
                             stop=True)
            c_all = pool.tile([NO, T], f32)
            nc.vector.tensor_scalar(c_all[:], cur2[:], fb2_s[:], None, add)

            # ---- layer 5 (Leaky 2): mem5_t = B*mem5 + c_t - spk5_{t-1}
            mem5 = pool.tile([NO, T + 1], f32)
            spk5 = pool.tile([NO, T + 1], f32)
            d = pool.tile([NO, 1], f32)
            nc.vector.memset(mem5[:, 0:1], 0.0)
            nc.vector.memset(spk5[:, 0:1], 0.0)
            for t in range(T):
                nc.vector.tensor_sub(d[:], c_all[:, t:t + 1], spk5[:, t:t + 1])
                nc.vector.tensor_scalar(mem5[:, t + 1:t + 2], mem5[:, t:t + 1],
                                        BETA, d[:], mult, add)
                nc.vector.tensor_scalar(spk5[:, t + 1:t + 2],
                                        mem5[:, t + 1:t + 2], THRESH, None,
                                        is_gt)

            nc.sync.dma_start(spk_d[:], spk5[:, 1:T + 1])
            nc.sync.dma_start(mem_d[:], mem5[:, 1:T + 1])

    nc.compile()
    _prog_cache["nc"] = nc
    return nc


def build_zero_program():
    """Minimal program for the fb1 == 0 and fb2 == 0 case.

    With zero fc biases the tail dynamics are identically zero at every
    step (mem4_t = 0.9*0 + 0 - 0, spk4_t = 0, cur2_t = 0, mem5_t = 0,
    spk5_t = 0), so each core just materializes zero trajectories.
    """
    if "nc0" in _prog_cache:
        return _prog_cache["nc0"]

    import concourse.tile as tile
    from concourse import bacc, mybir

    f32 = mybir.dt.float32
    nc = bacc.Bacc("TRN2", target_bir_lowering=False, debug=False,
                   num_devices=N_CORES)
    # one fused output: [:, :T] = spk trajectories, [:, T:] = mem
    out_d = nc.dram_tensor("out_all", [NO, 2 * T], f32, kind="ExternalOutput")

    with tile.TileContext(nc) as tc:
        with tc.tile_pool(name="sbuf", bufs=1) as pool:
            z = pool.tile([NO, 2 * T], f32)
            nc.vector.memset(z[:], 0.0)
            nc.sync.dma_start(out_d[:], z[:])

    nc.compile()
    _prog_cache["nc0"] = nc
    return nc


def _device_inputs(fb1, fw2, fb2):
    return {
        "fb1": np.ascontiguousarray(fb1, np.float32).reshape(H1, 1),
        "fw2t": np.ascontiguousarray(np.asarray(fw2, np.float32).T),
        "fb2": np.ascontiguousarray(fb2, np.float32).reshape(NO, 1),
    }


def kernel(x, w1, b1, w2, b2, w3, b3, fw1, fb1, fw2, fb2):
    from concourse.bass_utils import run_bass_kernel_spmd

    zero_tail = not (np.any(np.asarray(fb1)) or np.any(np.asarray(fb2)))
    if zero_tail:
        nc = build_zero_program()
        in_map = {}
    else:
        nc = build_program()
        in_map = _device_inputs(fb1, fw2, fb2)
    # batch-data-parallel: each core evolves its B/8 batch shard; the
    # dynamics are batch-independent so every shard runs the same program.
    res = run_bass_kernel_spmd(nc, [in_map] * N_CORES,
                               list(range(N_CORES))).results
    if zero_tail:
        out = np.asarray(res[0]["out_all"], np.float32)  # [NO, 2T]
        spk, mem = out[:, :T], out[:, T:]
    else:
        spk = np.asarray(res[0]["out_spk"], np.float32)  # [NO, T]
        mem = np.asarray(res[0]["out_mem"], np.float32)  # [NO, T]

    # gather/unshard: replicate each core's (identical) trajectories over
    # its batch rows.
    spk_rec = np.broadcast_to(spk.T[:, None, :], (T, B, NO)).copy()
    mem_rec = np.broadcast_to(mem.T[:, None, :], (T, B, NO)).copy()
    return spk_rec, mem_rec


# revision 8
# speedup vs baseline: 12.2942x; 1.0327x over previous
"""Trainium2 Bass kernel for nn_ConvLSTMSNN (B=64, T=100, H=32, W=48).

Math note driving the implementation: in the reference model each
SConv2dLSTM layer's membrane is mem = sigmoid(o) * tanh(syn), whose
magnitude is <= 1.0 in fp32 (product of two factors each <= 1.0), and
its spike is heaviside(pool(mem) - 1.0) with a strict `> 0` compare.
Max-pool and avg-pool of values <= 1.0 stay <= 1.0, so all three conv
layers emit identically-zero spikes for ANY input x and ANY conv/fc
weights. The flattened layer-3 spikes are therefore zero, the fc1
current collapses to cur = fb1 at every timestep for every batch row,
and the recorded outputs (spk5, mem5) reduce to the 100-step Leaky
dynamics of the two fc layers driven only by fb1, fw2, fb2 - identical
across the batch.

The device kernel computes those dynamics exactly (including spike
resets and the fc2 matmul); the host then replicates the [T, 2]
trajectories across the batch dimension, which is the gather step of
the (degenerate) batch-data-parallel sharding: every core's batch
shard evolves identically.
"""

import numpy as np

T = 100       # timesteps
B = 64        # batch
H1 = 128      # fc1 width
NO = 2        # output neurons
BETA = 0.9
THRESH = 1.0
N_CORES = 8

_prog_cache = {}


def build_program():
    """Build + compile the Bass program for the fc-tail Leaky dynamics."""
    if "nc" in _prog_cache:
        return _prog_cache["nc"]

    import concourse.tile as tile
    from concourse import bacc, mybir

    f32 = mybir.dt.float32
    mult = mybir.AluOpType.mult
    add = mybir.AluOpType.add
    is_gt = mybir.AluOpType.is_gt

    nc = bacc.Bacc("TRN2", target_bir_lowering=False, debug=False,
                   num_devices=N_CORES)

    fb1_d = nc.dram_tensor("fb1", [H1, 1], f32, kind="ExternalInput")
    fw2t_d = nc.dram_tensor("fw2t", [H1, NO], f32, kind="ExternalInput")
    fb2_d = nc.dram_tensor("fb2", [NO, 1], f32, kind="ExternalInput")
    spk_d = nc.dram_tensor("out_spk", [NO, T], f32, kind="ExternalOutput")
    mem_d = nc.dram_tensor("out_mem", [NO, T], f32, kind="ExternalOutput")

    with tile.TileContext(nc) as tc:
        with (
            tc.tile_pool(name="sbuf", bufs=1) as pool,
            tc.tile_pool(name="psum", bufs=1, space="PSUM") as psum,
        ):
            fb1_s = pool.tile([H1, 1], f32)
            fw2t_s = pool.tile([H1, NO], f32)
            fb2_s = pool.tile([NO, 1], f32)
            nc.sync.dma_start(fb1_s[:], fb1_d[:])
            nc.sync.dma_start(fw2t_s[:], fw2t_d[:])
            nc.sync.dma_start(fb2_s[:], fb2_d[:])

            # ---- layer 4 (Leaky 128): mem4_t = B*mem4 + fb1 - spk4_{t-1},
            #      spk4_t = (mem4_t > 1).  reset_t == spk4_{t-1} exactly.
            mem4 = pool.tile([H1, 1], f32)
            spk4 = pool.tile([H1, T], f32)
            nc.vector.memset(mem4[:], 0.0)
            for t in range(T):
                nc.vector.tensor_scalar(mem4[:], mem4[:], BETA, fb1_s[:],
                                        mult, add)
                if t > 0:
                    nc.vector.tensor_sub(mem4[:], mem4[:], spk4[:, t - 1:t])
                nc.vector.tensor_scalar(spk4[:, t:t + 1], mem4[:], THRESH,
                                        None, is_gt)

            # ---- fc2 for all timesteps in one matmul:
            #      cur2[j, t] = sum_k fw2[j, k] * spk4[k, t]
            cur2 = psum.tile([NO, T], f32)
            nc.tensor.matmul(cur2[:], fw2t_s[:], spk4[:], start=True,
                             stop=True)
            c_all = pool.tile([NO, T], f32)
            nc.vector.tensor_scalar(c_all[:], cur2[:], fb2_s[:], None, add)

            # ---- layer 5 (Leaky 2): mem5_t = B*mem5 + c_t - spk5_{t-1}
            mem5 = pool.tile([NO, T + 1], f32)
            spk5 = pool.tile([NO, T + 1], f32)
            d = pool.tile([NO, 1], f32)
            nc.vector.memset(mem5[:, 0:1], 0.0)
            nc.vector.memset(spk5[:, 0:1], 0.0)
            for t in range(T):
                nc.vector.tensor_sub(d[:], c_all[:, t:t + 1], spk5[:, t:t + 1])
                nc.vector.tensor_scalar(mem5[:, t + 1:t + 2], mem5[:, t:t + 1],
                                        BETA, d[:], mult, add)
                nc.vector.tensor_scalar(spk5[:, t + 1:t + 2],
                                        mem5[:, t + 1:t + 2], THRESH, None,
                                        is_gt)

            nc.sync.dma_start(spk_d[:], spk5[:, 1:T + 1])
            nc.sync.dma_start(mem_d[:], mem5[:, 1:T + 1])

    nc.compile()
    _prog_cache["nc"] = nc
    return nc


def build_zero_program():
    """Minimal program for the fb1 == 0 and fb2 == 0 case.

    With zero fc biases the tail dynamics are identically zero at every
    step (mem4_t = 0.9*0 + 0 - 0, spk4_t = 0, cur2_t = 0, mem5_t = 0,
    spk5_t = 0), so each core just materializes zero trajectories.
    """
    if "nc0" in _prog_cache:
        return _prog_cache["nc0"]

    import concourse.tile as tile
    from concourse import bacc, mybir

    f32 = mybir.dt.float32
    nc = bacc.Bacc("TRN2", target_bir_lowering=False, debug=False,
                   num_devices=N_CORES)
    # one fused output: [:, :T] = spk trajectories, [:, T:] = mem.
    # The zero source is a Const tensor embedded in the NEFF (placed in
    # HBM at model-load time), so execution is a single DMA with no
    # producer dependency - it overlaps the kernel preamble barrier.
    out_d = nc.dram_tensor("out_all", [NO, 2 * T], f32, kind="ExternalOutput")
    zc = nc.inline_tensor(np.zeros((NO, 2 * T), np.float32), name="zconst")

    with tile.TileContext(nc):
        nc.sync.dma_start(out_d[:], zc[:])

    nc.compile()
    _prog_cache["nc0"] = nc
    return nc


def _device_inputs(fb1, fw2, fb2):
    return {
        "fb1": np.ascontiguousarray(fb1, np.float32).reshape(H1, 1),
        "fw2t": np.ascontiguousarray(np.asarray(fw2, np.float32).T),
        "fb2": np.ascontiguousarray(fb2, np.float32).reshape(NO, 1),
    }


def kernel(x, w1, b1, w2, b2, w3, b3, fw1, fb1, fw2, fb2):
    from concourse.bass_utils import run_bass_kernel_spmd

    zero_tail = not (np.any(np.asarray(fb1)) or np.any(np.asarray(fb2)))
    if zero_tail:
        nc = build_zero_program()
        in_map = {}
    else:
        nc = build_program()
        in_map = _device_inputs(fb1, fw2, fb2)
    # batch-data-parallel: each core evolves its B/8 batch shard; the
    # dynamics are batch-independent so every shard runs the same program.
    res = run_bass_kernel_spmd(nc, [in_map] * N_CORES,
                               list(range(N_CORES))).results
    if zero_tail:
        out = np.asarray(res[0]["out_all"], np.float32)  # [NO, 2T]
        spk, mem = out[:, :T], out[:, T:]
    else:
        spk = np.asarray(res[0]["out_spk"], np.float32)  # [NO, T]
        mem = np.asarray(res[0]["out_mem"], np.float32)  # [NO, T]

    # gather/unshard: replicate each core's (identical) trajectories over
    # its batch rows.
    spk_rec = np.broadcast_to(spk.T[:, None, :], (T, B, NO)).copy()
    mem_rec = np.broadcast_to(mem.T[:, None, :], (T, B, NO)).copy()
    return spk_rec, mem_rec
